# revision 7
# baseline (speedup 1.0000x reference)
"""Trainium2 Bass kernel for BrainNetworkFusionModel (gnn_message_passing).

Sharding: graphs (subjects) are partitioned across the 8 NeuronCores -- core c
owns nodes [c*NN, (c+1)*NN).  Edges are bucketed by destination core on the
host, sorted into 128-node destination windows, and streamed on-device through
dma_gather (row gather by src) + one-hot matmuls that implement segment-sum in
PSUM.  Node features cross core boundaries twice via AllGather (after the
temporal encoder, and after the first GCN layer).  Small weight matrices are
replicated.  Mean-pooling and the predictor are device-local per graph.
"""
import numpy as np
import ml_dtypes

import concourse.bass as bass
import concourse.tile as tile
from concourse import bacc, mybir
from concourse.bass import AP
from concourse.bass_utils import run_bass_kernel_spmd

F32 = mybir.dt.float32
BF16 = mybir.dt.bfloat16
I16 = mybir.dt.int16
I32 = mybir.dt.int32
BF_NP = ml_dtypes.bfloat16

NCORES = 8
WINP = 128            # dst nodes per window
CALL = 1024           # indices per dma_gather call
SPLIT = 32768         # int16 gather index limit
BN_EPS = 1e-5
BN_SCALE = 1.0 / np.sqrt(1.0 + BN_EPS)
DEAD = 200.0          # dst-column sentinel for padded slots (outside [0,128))

_cache = {}
_last = None


# ----------------------------------------------------------------------------
# host-side preprocessing
# ----------------------------------------------------------------------------

def _gcn_norm(edge_index, edge_weight, n):
    src = np.concatenate([edge_index[0], np.arange(n, dtype=np.int64)])
    dst = np.concatenate([edge_index[1], np.arange(n, dtype=np.int64)])
    w = np.concatenate([edge_weight, np.ones(n, np.float32)]).astype(np.float64)
    deg = np.bincount(dst, weights=w, minlength=n)
    dinv = np.where(deg > 0, 1.0 / np.sqrt(deg), 0.0)
    return src, dst, (dinv[src] * w * dinv[dst]).astype(np.float32)


def _prep_modality(src, dst, wn, nn, core):
    """Bucket one modality's edges for one core. Returns (s, win, col, hi, w)
    sorted by (window, hi)."""
    base = core * nn
    sel = (dst >= base) & (dst < base + nn)
    s = src[sel]
    d = (dst[sel] - base).astype(np.int64)
    w = wn[sel]
    win = d >> 7
    col = d & 127
    hi = (s >= SPLIT).astype(np.int64)
    order = np.lexsort((hi, win))
    return s[order], win[order], col[order], hi[order], w[order]


def _pack_modality(s, win, col, hi, w, W, CL, CH):
    """Produce device arrays: idx16 [128, W*CT*64], cnts [W*CT] int32,
    dw [128, W*2*KMAX] bf16  (dw[:, w*2K:(w*2K+K)] = dcol, then wts)."""
    CT = CL + CH
    KMAX = 8 * CT
    ne = len(s)
    g = win * 2 + hi
    cnt = np.bincount(g, minlength=2 * W)
    starts = np.concatenate([[0], np.cumsum(cnt)])[:-1]
    rank = np.arange(ne, dtype=np.int64) - starts[g]
    assert (rank[hi == 0] < CL * CALL).all(), "lo capacity exceeded"
    assert (rank[hi == 1] < CH * CALL).all(), "hi capacity exceeded"
    slot = rank + hi * (CL * CALL)          # slot within window [0, CT*1024)
    callg = win * CT + (slot >> 10)         # global call id
    i = slot & 1023                         # position within call
    idxval = (s - hi * SPLIT).astype(np.int16)

    idx16 = np.full((16, W * CT * 64), -1, np.int16)
    idx16[i % 16, callg * 64 + (i >> 4)] = idxval
    cnts = np.bincount(callg, minlength=W * CT).astype(np.int32)
    # dummy entry for empty calls (ucode needs >= 1 valid index)
    empty = np.nonzero(cnts == 0)[0]
    idx16[0, empty * 64] = 0
    cnts[empty] = 1

    dcol = np.full((128, W * KMAX), DEAD, np.float32)
    wts = np.zeros((128, W * KMAX), np.float32)
    cw = win * KMAX + (slot >> 7)           # global chunk id
    p = slot & 127
    dcol[p, cw] = col
    wts[p, cw] = w

    dw = np.empty((128, W, 2, KMAX), np.float32)
    dw[:, :, 0, :] = dcol.reshape(128, W, KMAX)
    dw[:, :, 1, :] = wts.reshape(128, W, KMAX)
    return (np.tile(idx16, (8, 1)), cnts.reshape(1, -1),
            dw.reshape(128, W * 2 * KMAX).astype(BF_NP))


def _fuse_bn(gamma, beta, bias):
    """relu(bn(t + bias)) = relu(t*s + b) with s,b below."""
    s = gamma * BN_SCALE
    return s.astype(np.float32), (bias * s + beta).astype(np.float32)


# ----------------------------------------------------------------------------
# device program
# ----------------------------------------------------------------------------

def _bcast_mid(ap, k):
    """[128, F] -> [128, (0,k), F]"""
    return AP(ap.tensor, ap.offset, [list(ap.ap[0]), [0, k], list(ap.ap[1])])


def _bcast_last(ap, k):
    """[128, F] -> [128, F, (0,k)]"""
    return AP(ap.tensor, ap.offset, [list(ap.ap[0]), list(ap.ap[1]), [0, k]])


def build_program(cfg):
    N, NN, W, CL, CH, GLOC, NPG = (cfg[k] for k in
                                   ("N", "NN", "W", "CL", "CH", "GLOC", "NPG"))
    CT = CL + CH
    KMAX = 8 * CT
    IN_D = cfg["IN_D"]          # 400 (x feature dim)
    ICH = (IN_D + 127) // 128   # ic chunks for TE layer 1
    ND = cfg["ND"]              # 64 node_feature_dim
    H = cfg["H"]                # 128 hidden
    C = cfg["C"]                # classes
    ds = bass.ds

    nc = bacc.Bacc("TRN2", target_bir_lowering=False, debug=False,
                   num_devices=NCORES)

    # ---- inputs ----
    xt = nc.dram_tensor("xt", [W * ICH * 128, 128], F32, kind="ExternalInput")
    ts = bass.ts
    idx_in, dw_in, cnt_in = [], [], []
    for m in range(2):
        idx_in.append(nc.dram_tensor(f"idx{m}", [128, W * CT * 64], I16,
                                     kind="ExternalInput"))
        dw_in.append(nc.dram_tensor(f"dw{m}", [128, W * 2 * KMAX], BF16,
                                    kind="ExternalInput"))
        cnt_in.append(nc.dram_tensor(f"cnt{m}", [1, W * CT], I32,
                                     kind="ExternalInput"))
    PARAMS = dict(
        teW1=([128, ICH * 128], F32), tef1s=([128, 1], F32), tef1b=([128, 1], F32),
        teW2=([128, ND], F32), teb2r=([1, ND], F32),
        W0m0=([ND, 128], BF16), W0m1=([ND, 128], BF16),
        b0r0=([1, 128], F32), b0r1=([1, 128], F32),
        W1m0=([128, 128], BF16), W1m1=([128, 128], BF16),
        f1s0=([128, 1], F32), f1b0=([128, 1], F32),
        f1s1=([128, 1], F32), f1b1=([128, 1], F32),
        gW0=([128, 128], BF16), gW1=([128, 128], BF16), gb=([128, 1], F32),
        prW1=([128, 128], F32), prs1=([128, 1], F32), prb1=([128, 1], F32),
        prW2=([128, H // 2], F32), prs2=([H // 2, 1], F32), prb2=([H // 2, 1], F32),
        prW3=([H // 2, C], F32), prb3=([C, 1], F32),
    )
    pin = {k: nc.dram_tensor(k, sh, dt, kind="ExternalInput")
           for k, (sh, dt) in PARAMS.items()}
    out = nc.dram_tensor("out", [C, GLOC], F32, kind="ExternalOutput")

    GBUFS = CT + 2

    with tile.TileContext(nc) as tc:
        with tc.tile_pool(name="const", bufs=1) as cp, \
             tc.tile_pool(name="dram", bufs=1, space="DRAM") as dp:
            # persistent tiles
            pt = {}
            for k, (sh, dt) in PARAMS.items():
                pt[k] = cp.tile(sh, dt, name=k, tag=k)
                nc.sync.dma_start(pt[k][:], pin[k][:])
            iota16 = cp.tile([128, 128], I16)
            nc.gpsimd.iota(iota16[:], pattern=[[1, 128]], base=0,
                           channel_multiplier=0)
            iotab = cp.tile([128, 128], BF16)
            nc.vector.tensor_copy(iotab[:], iota16[:])
            ones1 = cp.tile([1, 128], F32)
            nc.vector.memset(ones1[:], 1.0)
            cnt_sb = [cp.tile([1, W * CT], I32, name=f"cntsb{m}", tag=f"cnt{m}") for m in range(2)]
            for m in range(2):
                nc.sync.dma_start(cnt_sb[m][:], cnt_in[m][:])
            pooled = cp.tile([128, GLOC], F32)

            T0own = dp.tile([NN, 128], BF16)
            T0 = dp.tile([N, 128], BF16)
            T1own = dp.tile([NN, 256], BF16)
            T1 = dp.tile([N, 256], BF16)
            HF = dp.tile([128, NN], BF16)

            cregs = [nc.gpsimd.alloc_register(f"cnt_r{j}") for j in range(CT)]

            # ---------------- temporal encoder ----------------
            with tc.tile_pool(name="te", bufs=2) as sp, \
                 tc.tile_pool(name="teps", bufs=2, space="PSUM") as pp:
                def te_body(i):
                    xv = xt.ap()[ds(i * (ICH * 128), ICH * 128), :] \
                        .rearrange("(a p) d -> p a d", p=128)
                    xtt = sp.tile([128, ICH, 128], F32, tag="xt")
                    nc.sync.dma_start(xtt[:], xv)
                    tps = pp.tile([128, 128], F32, tag="tps")
                    for k in range(ICH):
                        nc.tensor.matmul(tps[:], pt["teW1"][:, ts(k, 128)], xtt[:, k, :],
                                         start=(k == 0), stop=(k == ICH - 1))
                    t2 = sp.tile([128, 128], F32, tag="t2")
                    nc.scalar.activation(t2[:], tps[:],
                                         mybir.ActivationFunctionType.Relu,
                                         bias=pt["tef1b"][:], scale=pt["tef1s"][:])
                    ps2 = pp.tile([128, ND], F32, tag="ps2")
                    nc.tensor.matmul(ps2[:], t2[:], pt["teW2"][:],
                                     start=True, stop=False)
                    nc.tensor.matmul(ps2[:], ones1[:], pt["teb2r"][:],
                                     start=False, stop=True)
                    h0 = sp.tile([128, ND], BF16, tag="h0")
                    nc.scalar.copy(h0[:], ps2[:])
                    nc.sync.dma_start(T0own[ds(i * 128, 128), 0:ND], h0[:])
                with tc.For_i(0, W, 1) as i:
                    te_body(i)

            nc.gpsimd.collective_compute(
                "AllGather", mybir.AluOpType.bypass,
                replica_groups=[list(range(NCORES))],
                ins=[T0own.opt()], outs=[T0.opt()])

            # ---------------- conv layer 0 (both modalities) ----------------
            with tc.tile_pool(name="l0", bufs=2) as sp, \
                 tc.tile_pool(name="l0g", bufs=GBUFS) as gp, \
                 tc.tile_pool(name="l0ps", bufs=2, space="PSUM") as pp:
                # pre-fill gather slots with zeros (stale-NaN guard)
                for m in range(2):
                    for b in range(GBUFS):
                        t = gp.tile([128, 8, 128], BF16, name=f"gz{m}", tag=f"g{m}")
                        nc.vector.memset(t[:], 0.0)

                def l0_body(i):
                    for m in range(2):
                        idxt = sp.tile([128, CT * 64], I16, tag=f"ix{m}")
                        nc.sync.dma_start(idxt[:], idx_in[m][:, ds(i * (CT * 64), CT * 64)])
                        dwt = sp.tile([128, 2, KMAX], BF16, tag=f"dw{m}")
                        nc.sync.dma_start(
                            dwt[:], dw_in[m][:, ds(i * (2 * KMAX), 2 * KMAX)]
                            .rearrange("p (a k) -> p a k", a=2))
                        S = sp.tile([128, KMAX, 128], BF16, tag=f"S{m}")
                        nc.vector.tensor_tensor(
                            S[:], _bcast_mid(iotab[:, :], KMAX),
                            _bcast_last(dwt[:, 0, :], 128),
                            op=mybir.AluOpType.is_equal)
                        nc.vector.tensor_tensor(
                            S[:], S[:], _bcast_last(dwt[:, 1, :], 128),
                            op=mybir.AluOpType.mult)
                        gts = []
                        for j in range(CT):
                            nc.gpsimd.reg_load(cregs[j],
                                               cnt_sb[m][0:1, ds(i * CT + j, 1)])
                            view = T0[0:min(SPLIT, N), :] if j < CL else T0[SPLIT:N, :]
                            gt = gp.tile([128, 8, 128], BF16, name=f"gz{m}", tag=f"g{m}")
                            nc.gpsimd.dma_gather(
                                gt[:], view, idxt[:, j * 64:(j + 1) * 64],
                                CALL, cregs[j], 128, elem_step=128,
                                single_packet=False)
                            gts.append(gt)
                        agg = pp.tile([ND, 128], F32, tag="agg")
                        for c in range(KMAX):
                            nc.tensor.matmul(agg[:], gts[c // 8][:, c % 8, 0:ND],
                                             S[:, c, :], start=(c == 0),
                                             stop=(c == KMAX - 1))
                        aggsb = sp.tile([ND, 128], BF16, tag=f"ag{m}")
                        nc.scalar.copy(aggsb[:], agg[:])
                        ps4 = pp.tile([128, 128], F32, tag="ps4")
                        nc.tensor.matmul(ps4[:], aggsb[:], pt[f"W0m{m}"][:],
                                         start=True, stop=False)
                        nc.tensor.matmul(ps4[:], ones1[:], pt[f"b0r{m}"][:],
                                         start=False, stop=True)
                        h1 = sp.tile([128, 128], BF16, tag=f"h1{m}")
                        nc.scalar.activation(h1[:], ps4[:],
                                             mybir.ActivationFunctionType.Relu)
                        nc.sync.dma_start(
                            T1own[ds(i * 128, 128), m * 128:(m + 1) * 128], h1[:])
                with tc.For_i(0, W, 1) as i:
                    l0_body(i)

            nc.gpsimd.collective_compute(
                "AllGather", mybir.AluOpType.bypass,
                replica_groups=[list(range(NCORES))],
                ins=[T1own.opt()], outs=[T1.opt()])

            # ---------- conv layer 1 + gate + fusion (both modalities) ------
            with tc.tile_pool(name="l1", bufs=2) as sp, \
                 tc.tile_pool(name="l1g", bufs=GBUFS) as gp, \
                 tc.tile_pool(name="l1ps", bufs=2, space="PSUM") as pp:
                for m in range(2):
                    for b in range(GBUFS):
                        t = gp.tile([128, 8, 128], BF16, name=f"gz{m}", tag=f"g{m}")
                        nc.vector.memset(t[:], 0.0)

                def l1_body(i):
                    h2 = []
                    for m in range(2):
                        idxt = sp.tile([128, CT * 64], I16, tag=f"ix{m}")
                        nc.sync.dma_start(idxt[:], idx_in[m][:, ds(i * (CT * 64), CT * 64)])
                        dwt = sp.tile([128, 2, KMAX], BF16, tag=f"dw{m}")
                        nc.sync.dma_start(
                            dwt[:], dw_in[m][:, ds(i * (2 * KMAX), 2 * KMAX)]
                            .rearrange("p (a k) -> p a k", a=2))
                        S = sp.tile([128, KMAX, 128], BF16, tag=f"S{m}")
                        nc.vector.tensor_tensor(
                            S[:], _bcast_mid(iotab[:, :], KMAX),
                            _bcast_last(dwt[:, 0, :], 128),
                            op=mybir.AluOpType.is_equal)
                        nc.vector.tensor_tensor(
                            S[:], S[:], _bcast_last(dwt[:, 1, :], 128),
                            op=mybir.AluOpType.mult)
                        gts = []
                        for j in range(CT):
                            nc.gpsimd.reg_load(cregs[j],
                                               cnt_sb[m][0:1, ds(i * CT + j, 1)])
                            lo = j < CL
                            c0, c1 = (0, 128) if m == 0 else (128, 256)
                            view = (T1[0:min(SPLIT, N), c0:c1] if lo
                                    else T1[SPLIT:N, c0:c1])
                            gt = gp.tile([128, 8, 128], BF16, name=f"gz{m}", tag=f"g{m}")
                            nc.gpsimd.dma_gather(
                                gt[:], view, idxt[:, j * 64:(j + 1) * 64],
                                CALL, cregs[j], 128, elem_step=256,
                                single_packet=False)
                            gts.append(gt)
                        agg = pp.tile([128, 128], F32, tag="agg")
                        for c in range(KMAX):
                            nc.tensor.matmul(agg[:], gts[c // 8][:, c % 8, :],
                                             S[:, c, :], start=(c == 0),
                                             stop=(c == KMAX - 1))
                        aggsb = sp.tile([128, 128], BF16, tag=f"ag{m}")
                        nc.scalar.copy(aggsb[:], agg[:])
                        ps5 = pp.tile([128, 128], F32, tag="ps5")
                        nc.tensor.matmul(ps5[:], pt[f"W1m{m}"][:], aggsb[:],
                                         start=True, stop=True)
                        h2m = sp.tile([128, 128], BF16, tag=f"h2{m}")
                        nc.scalar.activation(h2m[:], ps5[:],
                                             mybir.ActivationFunctionType.Relu,
                                             bias=pt[f"f1b{m}"][:],
                                             scale=pt[f"f1s{m}"][:])
                        h2.append(h2m)
                    psg = pp.tile([128, 128], F32, tag="psg")
                    nc.tensor.matmul(psg[:], pt["gW0"][:], h2[0][:],
                                     start=True, stop=False)
                    nc.tensor.matmul(psg[:], pt["gW1"][:], h2[1][:],
                                     start=False, stop=True)
                    gt_ = sp.tile([128, 128], BF16, tag="gate")
                    nc.scalar.activation(gt_[:], psg[:],
                                         mybir.ActivationFunctionType.Sigmoid,
                                         bias=pt["gb"][:])
                    dif = sp.tile([128, 128], BF16, tag="dif")
                    nc.vector.tensor_sub(dif[:], h2[0][:], h2[1][:])
                    nc.vector.tensor_mul(dif[:], gt_[:], dif[:])
                    hf = sp.tile([128, 128], BF16, tag="hf")
                    nc.vector.tensor_add(hf[:], dif[:], h2[1][:])
                    nc.sync.dma_start(HF[:, ds(i * 128, 128)], hf[:])
                with tc.For_i(0, W, 1) as i:
                    l1_body(i)

            # ---------------- mean-pool + predictor ----------------
            with tc.tile_pool(name="pool", bufs=3) as sp, \
                 tc.tile_pool(name="pps", bufs=2, space="PSUM") as pp:
                for g in range(GLOC):
                    hfg = sp.tile([128, NPG], BF16, tag="hfg")
                    nc.sync.dma_start(hfg[:], HF[:, g * NPG:(g + 1) * NPG])
                    nc.vector.tensor_reduce(pooled[:, g:g + 1], hfg[:],
                                            axis=mybir.AxisListType.X,
                                            op=mybir.AluOpType.add)
                z1p = pp.tile([128, GLOC], F32, tag="z1p")
                nc.tensor.matmul(z1p[:], pt["prW1"][:], pooled[:],
                                 start=True, stop=True)
                z1 = sp.tile([128, GLOC], F32, tag="z1")
                nc.scalar.activation(z1[:], z1p[:],
                                     mybir.ActivationFunctionType.Relu,
                                     bias=pt["prb1"][:], scale=pt["prs1"][:])
                z2p = pp.tile([H // 2, GLOC], F32, tag="z2p")
                nc.tensor.matmul(z2p[:], pt["prW2"][:], z1[:],
                                 start=True, stop=True)
                z2 = sp.tile([H // 2, GLOC], F32, tag="z2")
                nc.scalar.activation(z2[:], z2p[:],
                                     mybir.ActivationFunctionType.Relu,
                                     bias=pt["prb2"][:], scale=pt["prs2"][:])
                z3p = pp.tile([C, GLOC], F32, tag="z3p")
                nc.tensor.matmul(z3p[:], pt["prW3"][:], z2[:],
                                 start=True, stop=True)
                z3 = sp.tile([C, GLOC], F32, tag="z3")
                nc.scalar.activation(z3[:], z3p[:],
                                     mybir.ActivationFunctionType.Identity,
                                     bias=pt["prb3"][:])
                nc.sync.dma_start(out.ap()[:, :], z3[:])

    nc.compile()
    return nc


# ----------------------------------------------------------------------------
# top-level kernel
# ----------------------------------------------------------------------------

def kernel(x, edge_index_sc, edge_weight_sc, edge_index_fc, edge_weight_fc,
           batch, params):
    x = np.asarray(x, np.float32)
    batch = np.asarray(batch, np.int64)
    p = {k: np.asarray(v, np.float32) for k, v in params.items()}
    N = x.shape[0]
    IN_D = x.shape[1]
    assert N % NCORES == 0
    NN = N // NCORES
    assert NN % WINP == 0
    W = NN // WINP
    NPG = int(np.bincount(batch).max())
    assert NN % NPG == 0
    GLOC = NN // NPG
    ND = p["te_W2"].shape[1]
    H = p["te_W1"].shape[1]
    C = p["pr_W3"].shape[1]
    assert H == 128 and ND <= 128

    mods = []
    for ei, ew in ((edge_index_sc, edge_weight_sc),
                   (edge_index_fc, edge_weight_fc)):
        src, dst, wn = _gcn_norm(np.asarray(ei, np.int64),
                                 np.asarray(ew, np.float32), N)
        mods.append([_prep_modality(src, dst, wn, NN, c) for c in range(NCORES)])

    # global call capacities (same program for all cores)
    def _cap(which):
        mx = 0
        for mod in mods:
            for (s, win, col, hi, w) in mod:
                sel = hi == which
                if sel.any():
                    mx = max(mx, int(np.bincount(win[sel], minlength=W).max()))
        return -(-mx // CALL)
    CL = max(_cap(0), 1)
    CH = _cap(1) if N > SPLIT else 0

    cfg = dict(N=N, NN=NN, W=W, CL=CL, CH=CH, GLOC=GLOC, NPG=NPG,
               IN_D=IN_D, ND=ND, H=H, C=C)
    key = tuple(sorted(cfg.items()))
    if key not in _cache:
        _cache[key] = build_program(cfg)
    nc = _cache[key]
    ICH = (IN_D + 127) // 128

    # ---- parameters (shared by all cores) ----
    teW1p = np.zeros((ICH * 128, 128), np.float32)
    teW1p[:IN_D] = p["te_W1"]
    teW1 = teW1p.reshape(ICH, 128, 128).transpose(1, 0, 2).reshape(128, ICH * 128)
    tef1s, tef1b = _fuse_bn(p["te_bn_g"], p["te_bn_b"], p["te_b1"])
    pin = dict(
        teW1=teW1, tef1s=tef1s.reshape(-1, 1), tef1b=tef1b.reshape(-1, 1),
        teW2=p["te_W2"], teb2r=p["te_b2"].reshape(1, -1),
        gb=p["gate_b"].reshape(-1, 1),
        gW0=p["gate_W"][:128].astype(BF_NP), gW1=p["gate_W"][128:].astype(BF_NP),
        prb3=p["pr_b3"].reshape(-1, 1),
    )
    for m, pre in ((0, "sc"), (1, "fc")):
        s0, b0 = _fuse_bn(p[f"bn_{pre}_g0"], p[f"bn_{pre}_b0"], p[f"{pre}_b0"])
        pin[f"W0m{m}"] = (p[f"{pre}_W0"] * s0[None, :]).astype(BF_NP)
        pin[f"b0r{m}"] = b0.reshape(1, -1)
        s1, b1 = _fuse_bn(p[f"bn_{pre}_g1"], p[f"bn_{pre}_b1"], p[f"{pre}_b1"])
        pin[f"W1m{m}"] = p[f"{pre}_W1"].astype(BF_NP)
        pin[f"f1s{m}"] = s1.reshape(-1, 1)
        pin[f"f1b{m}"] = b1.reshape(-1, 1)
    ps1, pb1 = _fuse_bn(p["pr_bn1_g"], p["pr_bn1_b"], p["pr_b1"])
    ps2, pb2 = _fuse_bn(p["pr_bn2_g"], p["pr_bn2_b"], p["pr_b2"])
    pin.update(prW1=p["pr_W1"] / NPG, prs1=ps1.reshape(-1, 1),
               prb1=pb1.reshape(-1, 1),
               prW2=p["pr_W2"], prs2=ps2.reshape(-1, 1), prb2=pb2.reshape(-1, 1),
               prW3=p["pr_W3"])

    # ---- per-core inputs ----
    in_maps = []
    for c in range(NCORES):
        im = dict(pin)
        xc = x[c * NN:(c + 1) * NN]                       # [NN, IN_D]
        xtc = np.zeros((W, ICH * 128, 128), np.float32)
        xtc[:, :IN_D, :] = xc.reshape(W, 128, IN_D).transpose(0, 2, 1)
        im["xt"] = xtc.reshape(W * ICH * 128, 128)
        for m in range(2):
            idx16, cnts, dw = _pack_modality(*mods[m][c], W, CL, CH)
            im[f"idx{m}"] = idx16
            im[f"cnt{m}"] = cnts
            im[f"dw{m}"] = dw
        in_maps.append(im)

    global _last
    _last = (nc, in_maps)
    res = run_bass_kernel_spmd(nc, in_maps, core_ids=list(range(NCORES)))
    outs = [r["out"] for r in res.results]                # each [C, GLOC]
    return np.concatenate([o.T for o in outs], axis=0).astype(np.float32)
